# revision 1
# baseline (speedup 1.0000x reference)
"""Trainium2 Bass kernel for nn_Discriminator (2-layer GRU + FC + sigmoid).

Strategy (8 NeuronCores, data-parallel over batch, B=128 -> 16 per core):
  - Transposed layout everywhere: hidden dim on the 128 SBUF partitions,
    (half, batch) on the free dim. No transposes needed anywhere.
  - Recurrent matmuls: stationary bf16 weight tiles (FWL) x moving h tiles;
    PSUM [128, 6*BL] holds r,z,n-gate preacts per step.
  - x-projections hoisted out of the recurrence: layer-0's is precomputed
    upfront (big efficient matmuls into SBUF rings); layer-1's is computed in
    chunks of CH steps from the layer-0 output ring, pipelined with the
    recurrence (layer 1 lags layer 0 by CH steps).
  - xproj/bias injection into PSUM via an identity-matmul (DMA cannot touch
    PSUM); biases folded into ring evictions via tensor_scalar_add.
  - Gate math: sigmoid/tanh on ScalarE, elementwise on VectorE, all bf16.
"""
import numpy as np
import ml_dtypes

import concourse.bass as bass
import concourse.tile as tile
from concourse import bacc, mybir
from concourse.bass_utils import run_bass_kernel_spmd
from concourse.masks import make_identity

BF = ml_dtypes.bfloat16
F8 = ml_dtypes.float8_e4m3
B, T, I, H = 128, 512, 256, 256
NCORES = 8
BL = B // NCORES          # batch per core = 16
NG = 6                    # 128-row gate chunks in 3H = 768
KH = 2                    # 128-row contraction chunks in H = 256
CH = 16                   # layer-1 xproj chunk length (in steps)
F32 = mybir.dt.float32
BF16 = mybir.dt.bfloat16
FP8 = mybir.dt.float8e4
AF = mybir.ActivationFunctionType


def _g_bank(g, chunk):
    return (g * chunk * BL * 4) // 2048


def _bank_first(g, chunk):
    return g == 0 or _g_bank(g, chunk) != _g_bank(g - 1, chunk)


def _bank_last(g, chunk, n_g=NG):
    return g == n_g - 1 or _g_bank(g, chunk) != _g_bank(g + 1, chunk)


def build_program(t_steps=T, chunk=CH, repeats=1):
    assert t_steps % chunk == 0
    nc = bacc.Bacc("TRN2", target_bir_lowering=False)
    TB = t_steps * BL

    # ---------------- DRAM I/O ----------------
    xT_d = nc.declare_dram_parameter("xT", [KH, 128, TB], BF16, isOutput=False)
    w_d = {}
    for name in ("wih0", "whh0", "wih1", "whh1"):
        dt = FP8 if name.startswith("whh") else BF16
        w_d[name] = nc.declare_dram_parameter(name, [KH, 128, NG * 128], dt,
                                              isOutput=False)
    wfc_d = nc.declare_dram_parameter("wfc", [KH, 128, 1], BF16, isOutput=False)
    # per-gate biases broadcast over (chunk, batch), injected via id-MMs
    bias0_d = nc.declare_dram_parameter("bias0", [128, chunk, NG * BL], BF16,
                                        isOutput=False)
    bias1_d = nc.declare_dram_parameter("bias1", [128, chunk, NG * BL], BF16,
                                        isOutput=False)
    bnh0_d = nc.declare_dram_parameter("bnh0", [128, KH * BL], BF16, isOutput=False)
    bnh1_d = nc.declare_dram_parameter("bnh1", [128, KH * BL], BF16, isOutput=False)
    bfc_d = nc.declare_dram_parameter("bfc", [1, 1], F32, isOutput=False)
    y_d = nc.declare_dram_parameter("y", [1, BL], F32, isOutput=True)

    with tile.TileContext(nc) as tc:
        with tc.tile_pool(name="big", bufs=1) as big:
            # ------------- persistent SBUF -------------
            xT_sb = big.tile([128, KH, TB], BF16)
            w_sb = {n: big.tile([128, KH, NG * 128],
                                FP8 if n.startswith("whh") else BF16,
                                name=f"w_{n}")
                    for n in w_d}
            wfc_sb = big.tile([128, KH, 1], BF16)
            bias0_sb = big.tile([128, chunk, NG * BL], BF16)
            bias1_sb = big.tile([128, chunk, NG * BL], BF16)
            bnh0_sb = big.tile([128, KH * BL], BF16)
            bnh1_sb = big.tile([128, KH * BL], BF16)
            bfc_sb = big.tile([1, 1], F32)
            ident = big.tile([128, 128], BF16)
            ring0_rz = big.tile([128, t_steps, 4 * BL], BF16)
            ring0_n = big.tile([128, t_steps, 2 * BL], BF16)
            ring1_rz = big.tile([128, 2, chunk, 4 * BL], BF16)
            ring1_n = big.tile([128, 2, chunk, 2 * BL], BF16)
            h0ring = big.tile([128, 2 * chunk, KH, BL], BF16)
            h1ring = big.tile([128, 2, KH, BL], BF16)
            h_init = big.tile([128, KH, BL], BF16)
            y_sb = big.tile([1, BL], F32)

            # ------------- input DMAs -------------
            for k in range(KH):
                nc.sync.dma_start(out=xT_sb[:, k, :], in_=xT_d[k])
                for n in w_d:
                    nc.sync.dma_start(out=w_sb[n][:, k, :], in_=w_d[n][k])
                nc.sync.dma_start(out=wfc_sb[:, k, :], in_=wfc_d[k])
            nc.sync.dma_start(out=bias0_sb[:], in_=bias0_d[:])
            nc.sync.dma_start(out=bias1_sb[:], in_=bias1_d[:])
            nc.sync.dma_start(out=bnh0_sb[:], in_=bnh0_d[:])
            nc.sync.dma_start(out=bnh1_sb[:], in_=bnh1_d[:])
            nc.sync.dma_start(out=bfc_sb[:], in_=bfc_d[:])
            make_identity(nc, ident[:])
            nc.vector.memset(h_init[:], 0.0)

            # ------------- PSUM pools (8 banks total) -------------
            with (
                tc.tile_pool(name="ps0", bufs=1, space=bass.MemorySpace.PSUM) as ps0p,
                tc.tile_pool(name="ps1", bufs=1, space=bass.MemorySpace.PSUM) as ps1p,
                tc.tile_pool(name="xp", bufs=2, space=bass.MemorySpace.PSUM) as xpp,
                tc.tile_pool(name="gates", bufs=6) as gates,
            ):
                HG = NG // 2  # g-chunks per xproj psum tile

                def emit_xproj_half(w_name, bias_sb, rhs, half, evict):
                    """MM 3 gate-chunks of an x-projection + biases via id-MM,
                    then evict each. evict(g, src [128, chunk, BL]) -> ring."""
                    xps = xpp.tile([128, HG, chunk, BL], F32, tag="xp",
                                   name="xps")
                    for gl in range(HG):
                        g = half * HG + gl
                        for k in range(KH):
                            nc.tensor.matmul(
                                xps[:, gl],
                                w_sb[w_name][:, k, 128 * g:128 * (g + 1)],
                                rhs[k],
                                start=(k == 0 and _bank_first(gl, chunk)),
                                stop=False,
                            )
                    for gl in range(HG):
                        g = half * HG + gl
                        nc.tensor.matmul(
                            xps[:, gl], ident[:],
                            bias_sb[:, :, g * BL:(g + 1) * BL],
                            start=False, stop=_bank_last(gl, chunk, HG),
                        )
                    for gl in range(HG):
                        evict(half * HG + gl, xps[:, gl])

                def evict0(nt):
                    trange = slice(nt * chunk, (nt + 1) * chunk)

                    def ev(g, src):
                        if g < 4:
                            dest = ring0_rz[:, trange, g * BL:(g + 1) * BL]
                        else:
                            dest = ring0_n[:, trange, (g - 4) * BL:(g - 3) * BL]
                        nc.vector.tensor_copy(dest, src)
                    return ev

                # ------------- phase 2 helpers -------------

                def emit_step(l, t):
                    whh = w_sb["whh0"] if l == 0 else w_sb["whh1"]
                    bnh = bnh0_sb if l == 0 else bnh1_sb
                    pool = ps0p if l == 0 else ps1p
                    # rz and v preacts live in SEPARATE banks so the sigmoid
                    # (ACT read of rz) can overlap the v matmuls (PE write) —
                    # same-bank PE-W + ACT-R would be serialized by Tile.
                    ps = pool.tile([128, 4 * BL], F32, tag=f"ps{l}")
                    pv = pool.tile([128, 2 * BL], F32, tag=f"pv{l}")
                    if l == 0:
                        rz_src = ring0_rz[:, t, :]
                        n_src = ring0_n[:, t, :]
                        h_prev = (h_init[:] if t == 0
                                  else h0ring[:, (t - 1) % (2 * chunk)])
                        h_new = h0ring[:, t % (2 * chunk)]
                    else:
                        cb = (t // chunk) % 2
                        rz_src = ring1_rz[:, cb, t % chunk, :]
                        n_src = ring1_n[:, cb, t % chunk, :]
                        h_prev = h_init[:] if t == 0 else h1ring[:, (t - 1) % 2]
                        h_new = h1ring[:, t % 2]
                    # both identity-MMs first (consecutive ident LDWs),
                    # then the rz group so the sigmoid unblocks earliest
                    nc.tensor.matmul(ps[:], ident[:], rz_src,
                                     start=True, stop=False)
                    nc.tensor.matmul(pv[:], ident[:], bnh[:],
                                     start=True, stop=False)
                    for g in range(4):
                        for k in range(KH):
                            nc.tensor.matmul(
                                ps[:, g * BL:(g + 1) * BL],
                                whh[:, k, 128 * g:128 * (g + 1)],
                                h_prev[:, k, :],
                                start=False,
                                stop=(g == 3 and k == KH - 1),
                            )
                    for g in range(4, NG):
                        for k in range(KH):
                            nc.tensor.matmul(
                                pv[:, (g - 4) * BL:(g - 3) * BL],
                                whh[:, k, 128 * g:128 * (g + 1)],
                                h_prev[:, k, :],
                                start=False,
                                stop=(g == NG - 1 and k == KH - 1),
                            )
                    rz = gates.tile([128, 4 * BL], BF16, tag=f"rz{l}")
                    nc.scalar.activation(rz[:], ps[:], AF.Sigmoid)
                    rv = gates.tile([128, 2 * BL], BF16, tag=f"rv{l}")
                    nc.vector.tensor_mul(rv[:], rz[:, 0:2 * BL], pv[:])
                    av = gates.tile([128, 2 * BL], BF16, tag=f"a{l}")
                    nc.vector.tensor_add(av[:], rv[:], n_src)
                    nn = gates.tile([128, 2 * BL], BF16, tag=f"n{l}")
                    nc.scalar.activation(nn[:], av[:], AF.Tanh)
                    dd = gates.tile([128, KH, BL], BF16, tag=f"d{l}")
                    nc.vector.tensor_sub(dd[:], h_prev, nn[:])
                    ee = gates.tile([128, KH, BL], BF16, tag=f"e{l}")
                    nc.vector.tensor_mul(ee[:], rz[:, 2 * BL:4 * BL], dd[:])
                    nc.vector.tensor_add(h_new, nn[:], ee[:])

                def emit_chunk(kc):
                    # layer-1 xproj for steps [kc*chunk, (kc+1)*chunk)
                    cb = kc % 2
                    s0 = (kc * chunk) % (2 * chunk)
                    rhs = [h0ring[:, s0:s0 + chunk, k, :] for k in range(KH)]

                    def ev(g, src):
                        if g < 4:
                            dest = ring1_rz[:, cb, :, g * BL:(g + 1) * BL]
                        else:
                            dest = ring1_n[:, cb, :, (g - 4) * BL:(g - 3) * BL]
                        nc.vector.tensor_copy(dest, src)

                    for half in range(2):
                        emit_xproj_half("wih1", bias1_sb, rhs, half, ev)

                for _rep in range(repeats):
                    # ---------- phase 1: xproj0 precompute ----------
                    for nt in range(t_steps // chunk):
                        cols = slice(nt * chunk * BL, (nt + 1) * chunk * BL)
                        rhs = [xT_sb[:, k, cols] for k in range(KH)]
                        for half in range(2):
                            emit_xproj_half("wih0", bias0_sb, rhs, half,
                                            evict0(nt))
                    # ---------- phase 2: recurrence ----------
                    for s in range(t_steps + chunk):
                        if s < t_steps:
                            emit_step(0, s)
                            if (s + 1) % chunk == 0:
                                emit_chunk(s // chunk)
                        if s >= chunk:
                            emit_step(1, s - chunk)

                    # ---------- head: y = sigmoid(h1 @ WfcT + bfc) ----------
                    hps = ps0p.tile([1, BL], F32, tag="ps0", name="hps")
                    hfin = h1ring[:, (t_steps - 1) % 2]
                    for k in range(KH):
                        nc.tensor.matmul(hps[:], wfc_sb[:, k, :],
                                         hfin[:, k, :],
                                         start=(k == 0), stop=(k == KH - 1))
                    nc.scalar.activation(y_sb[:], hps[:], AF.Sigmoid,
                                         bias=bfc_sb[:])
                    nc.sync.dma_start(out=y_d[:], in_=y_sb[:])

    nc.finalize()
    return nc


# ---------------- host-side prep ----------------

def _wT_tiles(w, dt=BF):
    """[3H, D] fp32 -> [KH, 128, 3H] (transposed, K-chunked)."""
    wt = np.ascontiguousarray(w.T)                      # [D, 3H]
    return wt.reshape(KH, 128, w.shape[0]).astype(dt)


def _prep_shared(Wih0, Whh0, bih0, bhh0, Wih1, Whh1, bih1, bhh1, Wfc, bfc,
                 chunk=CH):
    out = {
        "wih0": _wT_tiles(Wih0), "whh0": _wT_tiles(Whh0, F8),
        "wih1": _wT_tiles(Wih1), "whh1": _wT_tiles(Whh1, F8),
        "wfc": np.ascontiguousarray(Wfc.T).reshape(KH, 128, 1).astype(BF),
        "bfc": np.asarray(bfc, np.float32).reshape(1, 1),
    }
    for l, (bi, bh) in enumerate(((bih0, bhh0), (bih1, bhh1))):
        bias = np.empty((128, NG), np.float32)
        for g in range(NG):
            rows = slice(128 * g, 128 * (g + 1))
            bias[:, g] = bi[rows] + (bh[rows] if g < 4 else 0.0)
        # broadcast to [128, chunk, NG*BL] for the id-MM injection
        bc = np.broadcast_to(bias[:, None, :, None], (128, chunk, NG, BL))
        out[f"bias{l}"] = np.ascontiguousarray(
            bc.reshape(128, chunk, NG * BL)).astype(BF)
        bnh = bh[2 * H:3 * H].astype(np.float32)         # [256]
        bnh_bc = np.repeat(bnh.reshape(KH, 128).transpose(1, 0)[:, :, None],
                           BL, axis=2)                   # [128, KH, BL]
        out[f"bnh{l}"] = bnh_bc.reshape(128, KH * BL).astype(BF)
    return out


def _prep_x_core(x_c, t_steps=T):
    """[BL, T, I] fp32 -> [KH, 128, T*BL] bf16 (xT[k,p,t*BL+b]=x[b,t,128k+p])."""
    xt = x_c.transpose(2, 1, 0)                          # [I, T, BL]
    return np.ascontiguousarray(xt.reshape(KH, 128, t_steps * BL)).astype(BF)


def make_in_maps(inputs, t_steps=T, chunk=CH):
    shared = _prep_shared(
        inputs["Wih0"], inputs["Whh0"], inputs["bih0"], inputs["bhh0"],
        inputs["Wih1"], inputs["Whh1"], inputs["bih1"], inputs["bhh1"],
        inputs["Wfc"], inputs["bfc"], chunk)
    x = np.asarray(inputs["x"], np.float32)
    in_maps = []
    for c in range(NCORES):
        m = dict(shared)
        m["xT"] = _prep_x_core(x[c * BL:(c + 1) * BL], t_steps)
        in_maps.append(m)
    return in_maps


_PROG_CACHE = {}


def run(inputs, t_steps=T, chunk=CH, trace=False):
    key = (t_steps, chunk)
    if key not in _PROG_CACHE:
        _PROG_CACHE[key] = build_program(t_steps, chunk)
    nc = _PROG_CACHE[key]
    in_maps = make_in_maps(inputs, t_steps, chunk)
    res = run_bass_kernel_spmd(nc, in_maps, list(range(NCORES)), trace=trace)
    y = np.concatenate([np.asarray(r["y"], np.float32).reshape(BL)
                        for r in res.results])
    return y.reshape(B, 1), res


def kernel(**inputs):
    y, _ = run(inputs)
    return y



# revision 8
# speedup vs baseline: 1727.0657x; 1727.0657x over previous
"""Trainium2 Bass kernel for nn_Discriminator (2-layer GRU + FC + sigmoid).

Strategy (8 NeuronCores, data-parallel over batch, B=128 -> 16 per core):
  - Transposed layout everywhere: hidden dim on the 128 SBUF partitions,
    (half, batch) on the free dim. No transposes needed anywhere.
  - Recurrent matmuls: stationary fp8 weight tiles (FWL) x moving h tiles;
    one PSUM bank per layer per step holds all 6 gate preacts [128, 96].
  - Per-slot emission is op-level interleaved across the two layers
    (engine queues are in-order; interleaving removes head-of-line
    blocking between the two recurrence streams -> ~1.7x).
  - PSUM pools double-buffered (bufs=2) so next step's xproj/bias
    injections run during the current step's gate math.
  - x-projections hoisted: layer-0's precomputed upfront into full-T
    rings; layer-1's computed chunk-wise from the layer-0 output ring,
    pipelined with the recurrence (layer 1 lags layer 0 by CH steps).
  - xproj injection into PSUM via fp8 identity-matmul; r/z/n input
    biases folded into ring evictions via tensor_scalar_add ([128,1]
    per-gate-chunk bias vectors) -- no bias id-matmuls anywhere.
  - Gate math: sigmoid/tanh on ScalarE, elementwise on VectorE, all bf16.
"""
import numpy as np
import ml_dtypes

import concourse.bass as bass
import concourse.tile as tile
from concourse import bacc, mybir
from concourse.bass_utils import run_bass_kernel_spmd
from concourse.masks import make_identity

BF = ml_dtypes.bfloat16
F8 = ml_dtypes.float8_e4m3
B, T, I, H = 128, 512, 256, 256
NCORES = 8
BL = B // NCORES          # batch per core = 16
NG = 6                    # 128-row gate chunks in 3H = 768
KH = 2                    # 128-row contraction chunks in H = 256
CH = 16                   # layer-1 xproj chunk length (in steps)
LAGC = 1                  # layer-1 lag in chunks
F32 = mybir.dt.float32
BF16 = mybir.dt.bfloat16
FP8 = mybir.dt.float8e4
AF = mybir.ActivationFunctionType


def build_program(t_steps=T, chunk=CH, repeats=1):
    assert t_steps % chunk == 0
    nc = bacc.Bacc("TRN2", target_bir_lowering=False)
    TB = t_steps * BL

    # ---------------- DRAM I/O ----------------
    xT_d = nc.declare_dram_parameter("xT", [KH, 128, TB], BF16, isOutput=False)
    w_d = {}
    for name in ("wih0", "whh0", "wih1", "whh1"):
        dt = FP8 if name.startswith("whh") else BF16
        w_d[name] = nc.declare_dram_parameter(name, [KH, 128, NG * 128], dt,
                                              isOutput=False)
    wfc_d = nc.declare_dram_parameter("wfc", [KH, 128, 1], BF16, isOutput=False)
    # per-gate-chunk biases [128, NG] fp32 (bi+bh for r/z chunks, bi for n)
    biasv0_d = nc.declare_dram_parameter("biasv0", [128, NG], F32, isOutput=False)
    biasv1_d = nc.declare_dram_parameter("biasv1", [128, NG], F32, isOutput=False)
    bnh0_d = nc.declare_dram_parameter("bnh0", [128, KH * BL], BF16, isOutput=False)
    bnh1_d = nc.declare_dram_parameter("bnh1", [128, KH * BL], BF16, isOutput=False)
    bfc_d = nc.declare_dram_parameter("bfc", [1, 1], F32, isOutput=False)
    y_d = nc.declare_dram_parameter("y", [1, BL], F32, isOutput=True)

    with tile.TileContext(nc) as tc:
        with tc.tile_pool(name="big", bufs=1) as big:
            # ------------- persistent SBUF -------------
            xT_sb = big.tile([128, KH, TB], BF16)
            w_sb = {n: big.tile([128, KH, NG * 128],
                                FP8 if n.startswith("whh") else BF16,
                                name=f"w_{n}")
                    for n in w_d}
            wfc_sb = big.tile([128, KH, 1], BF16)
            biasv0_sb = big.tile([128, NG], F32)
            biasv1_sb = big.tile([128, NG], F32)
            bnh0_sb = big.tile([128, KH * BL], BF16)
            bnh1_sb = big.tile([128, KH * BL], BF16)
            bfc_sb = big.tile([1, 1], F32)
            ident = big.tile([128, 128], BF16)
            ring0_rz = big.tile([128, t_steps, 4 * BL], BF16)
            ring0_n = big.tile([128, t_steps, 2 * BL], BF16)
            ring1_rz = big.tile([128, 2, chunk, 4 * BL], BF16)
            ring1_n = big.tile([128, 2, chunk, 2 * BL], BF16)
            h0ring = big.tile([128, 3 * chunk, KH, BL], BF16)
            h1ring = big.tile([128, 2, KH, BL], BF16)
            h_init = big.tile([128, KH, BL], BF16)
            y_sb = big.tile([1, BL], F32)

            # ------------- input DMAs -------------
            for k in range(KH):
                nc.sync.dma_start(out=xT_sb[:, k, :], in_=xT_d[k])
                for n in w_d:
                    nc.sync.dma_start(out=w_sb[n][:, k, :], in_=w_d[n][k])
                nc.sync.dma_start(out=wfc_sb[:, k, :], in_=wfc_d[k])
            nc.sync.dma_start(out=biasv0_sb[:], in_=biasv0_d[:])
            nc.sync.dma_start(out=biasv1_sb[:], in_=biasv1_d[:])
            nc.sync.dma_start(out=bnh0_sb[:], in_=bnh0_d[:])
            nc.sync.dma_start(out=bnh1_sb[:], in_=bnh1_d[:])
            nc.sync.dma_start(out=bfc_sb[:], in_=bfc_d[:])
            make_identity(nc, ident[:])
            nc.vector.memset(h_init[:], 0.0)

            # ------------- PSUM pools -------------
            # per-layer [128, 96] f32: rz preacts in [0:64], n preacts in
            # [64:96]; one bank per buffer, double-buffered so next step's
            # injections overlap this step's gate math.
            with (
                tc.tile_pool(name="pp0", bufs=2, space=bass.MemorySpace.PSUM) as pp0,
                tc.tile_pool(name="pp1", bufs=2, space=bass.MemorySpace.PSUM) as pp1,
                tc.tile_pool(name="gates", bufs=6) as gates,
            ):
                pools = (pp0, pp1)
                # xproj tiles share the recurrence ps/pv tags (8-bank budget)
                xtags = [(pp0, "ps0"), (pp0, "pv0"), (pp1, "ps1"), (pp1, "pv1")]

                def emit_xproj(w_name, bias_sb, rhs, evict):
                    """Per gate-chunk: accumulate K-chunk matmuls into a
                    [128, chunk*BL] PSUM tile, then evict with the bias
                    folded in (tensor_scalar_add with [128,1] bias)."""
                    for g in range(NG):
                        xpool, xtag = xtags[g % 4]
                        xg = xpool.tile([128, chunk * BL], F32, tag=xtag,
                                        name="xg")
                        for k in range(KH):
                            nc.tensor.matmul(
                                xg[:],
                                w_sb[w_name][:, k, 128 * g:128 * (g + 1)],
                                rhs[k],
                                start=(k == 0), stop=(k == KH - 1),
                            )
                        evict(g, xg, bias_sb[:, g:g + 1])

                def evict0(nt):
                    trange = slice(nt * chunk, (nt + 1) * chunk)

                    def ev(g, src, bias):
                        if g < 4:
                            dest = ring0_rz[:, trange, g * BL:(g + 1) * BL]
                        else:
                            dest = ring0_n[:, trange, (g - 4) * BL:(g - 3) * BL]
                        nc.vector.tensor_scalar_add(
                            dest, src.rearrange(f"p (c b) -> p c b", c=chunk),
                            bias)
                    return ev

                def chunk_work(kc):
                    """Work items (closures) for layer-1's xproj of chunk kc:
                    12 matmuls + 6 evictions, spread over the following
                    chunk's slots. mm items return the xg tile for g."""
                    cb = kc % 2
                    s0 = (kc * chunk) % (3 * chunk)
                    rhs = [h0ring[:, s0:s0 + chunk, k, :] for k in range(KH)]
                    xgs = {}
                    mms, evs = [], []

                    def mk_mm(g, k):
                        def it():
                            if k == 0:
                                xpool, xtag = xtags[g % 4]
                                xgs[g] = xpool.tile([128, chunk * BL], F32,
                                                    tag=xtag, name="xg")
                            nc.tensor.matmul(
                                xgs[g][:],
                                w_sb["wih1"][:, k, 128 * g:128 * (g + 1)],
                                rhs[k],
                                start=(k == 0), stop=(k == KH - 1))
                        return it

                    def mk_ev(g):
                        def it():
                            if g < 4:
                                dest = ring1_rz[:, cb, :, g * BL:(g + 1) * BL]
                            else:
                                dest = ring1_n[:, cb, :,
                                               (g - 4) * BL:(g - 3) * BL]
                            nc.vector.tensor_scalar_add(
                                dest,
                                xgs[g].rearrange("p (c b) -> p c b", c=chunk),
                                biasv1_sb[:, g:g + 1])
                        return it

                    for g in range(NG):
                        mms.append(mk_mm(g, 0))
                        mms.append(mk_mm(g, 1))
                        evs.append(mk_ev(g))
                    return mms, evs

                # ------------- recurrence slot (both layers zipped) -------
                def emit_slot(s, extra_mm=(), extra_ev=()):
                    lag = LAGC * chunk
                    ls = []
                    if s < t_steps:
                        ls.append(0)
                    if s >= lag:
                        ls.append(1)
                    st = {0: s, 1: s - lag}
                    ps, pv, rz_src, n_src, h_prev, h_new = {}, {}, {}, {}, {}, {}
                    whh = {0: w_sb["whh0"], 1: w_sb["whh1"]}
                    bnh = {0: bnh0_sb, 1: bnh1_sb}
                    for l in ls:
                        t = st[l]
                        if l == 0:
                            rz_src[l] = ring0_rz[:, t, :]
                            n_src[l] = ring0_n[:, t, :]
                            h_prev[l] = (h_init[:] if t == 0
                                         else h0ring[:, (t - 1) % (3 * chunk)])
                            h_new[l] = h0ring[:, t % (3 * chunk)]
                        else:
                            cb = (t // chunk) % 2
                            rz_src[l] = ring1_rz[:, cb, t % chunk, :]
                            n_src[l] = ring1_n[:, cb, t % chunk, :]
                            h_prev[l] = (h_init[:] if t == 0
                                         else h1ring[:, (t - 1) % 2])
                            h_new[l] = h1ring[:, t % 2]
                    # PSUM allocs + injections (independent of h -> run early)
                    for l in ls:
                        ps[l] = pools[l].tile([128, 4 * BL], F32,
                                              tag=f"ps{l}", name=f"ps{l}")
                        pv[l] = pools[l].tile([128, 2 * BL], F32,
                                              tag=f"pv{l}", name=f"pv{l}")
                        nc.tensor.matmul(ps[l][:], ident[:], rz_src[l],
                                         start=True, stop=False)
                        nc.tensor.matmul(pv[l][:], ident[:], bnh[l][:],
                                         start=True, stop=False)
                    # rz recurrent matmuls first so sigmoid unblocks earliest
                    for l in ls:
                        for g in range(4):
                            for k in range(KH):
                                nc.tensor.matmul(
                                    ps[l][:, g * BL:(g + 1) * BL],
                                    whh[l][:, k, 128 * g:128 * (g + 1)],
                                    h_prev[l][:, k, :],
                                    start=False, stop=(g == 3 and k == KH - 1))
                    for l in ls:
                        for g in range(4, NG):
                            for k in range(KH):
                                nc.tensor.matmul(
                                    pv[l][:, (g - 4) * BL:(g - 3) * BL],
                                    whh[l][:, k, 128 * g:128 * (g + 1)],
                                    h_prev[l][:, k, :],
                                    start=False,
                                    stop=(g == NG - 1 and k == KH - 1))
                    for it in extra_mm:
                        it()
                    rz, rv, av, nn = {}, {}, {}, {}
                    for l in ls:
                        rz[l] = gates.tile([128, 4 * BL], BF16, tag=f"rz{l}",
                                           name=f"rz{l}")
                        nc.scalar.activation(rz[l][:], ps[l][:], AF.Sigmoid)
                    for l in ls:
                        rv[l] = gates.tile([128, 2 * BL], BF16, tag=f"rv{l}",
                                           name=f"rv{l}")
                        nc.vector.tensor_mul(rv[l][:], rz[l][:, 0:2 * BL],
                                             pv[l][:])
                        av[l] = gates.tile([128, 2 * BL], BF16, tag=f"a{l}",
                                           name=f"a{l}")
                        nc.vector.tensor_add(av[l][:], rv[l][:], n_src[l])
                    for l in ls:
                        nn[l] = gates.tile([128, 2 * BL], BF16, tag=f"n{l}",
                                           name=f"n{l}")
                        nc.scalar.activation(nn[l][:], av[l][:], AF.Tanh)
                    for l in ls:
                        dd = gates.tile([128, KH, BL], BF16, tag=f"d{l}",
                                        name=f"d{l}")
                        nc.vector.tensor_sub(dd[:], h_prev[l], nn[l][:])
                        ee = gates.tile([128, KH, BL], BF16, tag=f"e{l}",
                                        name=f"e{l}")
                        nc.vector.tensor_mul(ee[:], rz[l][:, 2 * BL:4 * BL],
                                             dd[:])
                        nc.vector.tensor_add(h_new[l], nn[l][:], ee[:])
                    for it in extra_ev:
                        it()

                for _rep in range(repeats):
                    # ---------- phase 1: xproj0 precompute ----------
                    for nt in range(t_steps // chunk):
                        cols = slice(nt * chunk * BL, (nt + 1) * chunk * BL)
                        rhs = [xT_sb[:, k, cols] for k in range(KH)]
                        emit_xproj("wih0", biasv0_sb, rhs, evict0(nt))
                    # ---------- phase 2: recurrence ----------
                    lag = LAGC * chunk
                    for s in range(t_steps + lag):
                        emit_slot(s)
                        if s < t_steps and (s + 1) % chunk == 0:
                            mms, evs = chunk_work(s // chunk)
                            for it in mms:
                                it()
                            for it in evs:
                                it()

                    # ---------- head: y = sigmoid(h1 @ WfcT + bfc) ----------
                    hps = pp0.tile([1, BL], F32, tag="ps0", name="hps")
                    hfin = h1ring[:, (t_steps - 1) % 2]
                    for k in range(KH):
                        nc.tensor.matmul(hps[:], wfc_sb[:, k, :],
                                         hfin[:, k, :],
                                         start=(k == 0), stop=(k == KH - 1))
                    nc.scalar.activation(y_sb[:], hps[:], AF.Sigmoid,
                                         bias=bfc_sb[:])
                    nc.sync.dma_start(out=y_d[:], in_=y_sb[:])

    nc.finalize()
    return nc


# ---------------- host-side prep ----------------

def _wT_tiles(w, dt=BF):
    """[3H, D] fp32 -> [KH, 128, 3H] (transposed, K-chunked)."""
    wt = np.ascontiguousarray(w.T)                      # [D, 3H]
    return wt.reshape(KH, 128, w.shape[0]).astype(dt)


def _prep_shared(Wih0, Whh0, bih0, bhh0, Wih1, Whh1, bih1, bhh1, Wfc, bfc,
                 chunk=CH):
    out = {
        "wih0": _wT_tiles(Wih0), "whh0": _wT_tiles(Whh0, F8),
        "wih1": _wT_tiles(Wih1), "whh1": _wT_tiles(Whh1, F8),
        "wfc": np.ascontiguousarray(Wfc.T).reshape(KH, 128, 1).astype(BF),
        "bfc": np.asarray(bfc, np.float32).reshape(1, 1),
    }
    for l, (bi, bh) in enumerate(((bih0, bhh0), (bih1, bhh1))):
        bias = np.empty((128, NG), np.float32)
        for g in range(NG):
            rows = slice(128 * g, 128 * (g + 1))
            bias[:, g] = bi[rows] + (bh[rows] if g < 4 else 0.0)
        out[f"biasv{l}"] = bias
        bnh = bh[2 * H:3 * H].astype(np.float32)         # [256]
        bnh_bc = np.repeat(bnh.reshape(KH, 128).transpose(1, 0)[:, :, None],
                           BL, axis=2)                   # [128, KH, BL]
        out[f"bnh{l}"] = bnh_bc.reshape(128, KH * BL).astype(BF)
    return out


def _prep_x_core(x_c, t_steps=T):
    """[BL, T, I] fp32 -> [KH, 128, T*BL] bf16 (xT[k,p,t*BL+b]=x[b,t,128k+p])."""
    xt = x_c.transpose(2, 1, 0)                          # [I, T, BL]
    return np.ascontiguousarray(xt.reshape(KH, 128, t_steps * BL)).astype(BF)


def make_in_maps(inputs, t_steps=T, chunk=CH):
    shared = _prep_shared(
        inputs["Wih0"], inputs["Whh0"], inputs["bih0"], inputs["bhh0"],
        inputs["Wih1"], inputs["Whh1"], inputs["bih1"], inputs["bhh1"],
        inputs["Wfc"], inputs["bfc"], chunk)
    x = np.asarray(inputs["x"], np.float32)
    in_maps = []
    for c in range(NCORES):
        m = dict(shared)
        m["xT"] = _prep_x_core(x[c * BL:(c + 1) * BL], t_steps)
        in_maps.append(m)
    return in_maps


_PROG_CACHE = {}


def run(inputs, t_steps=T, chunk=CH, trace=False):
    key = (t_steps, chunk)
    if key not in _PROG_CACHE:
        _PROG_CACHE[key] = build_program(t_steps, chunk)
    nc = _PROG_CACHE[key]
    in_maps = make_in_maps(inputs, t_steps, chunk)
    res = run_bass_kernel_spmd(nc, in_maps, list(range(NCORES)), trace=trace)
    y = np.concatenate([np.asarray(r["y"], np.float32).reshape(BL)
                        for r in res.results])
    return y.reshape(B, 1), res


def kernel(**inputs):
    y, _ = run(inputs)
    return y
